# revision 1
# baseline (speedup 1.0000x reference)
"""Trainium2 Bass kernel for nn_CascadedAttention_76836964925817.

Math: the reference module's attention machinery is dead code — softmax over a
size-1 axis is identically 1, so `context = x[0].sum(axis=0)` is a constant
and the layer reduces to the 28-dim nonlinear recurrence

    y[t] = sigmoid(Wo @ y[t-1] + Uo @ x[t-1] + c),   c = Co @ sum_t x[t],
    y[-1] = 0, x[-1] := 0.

Strategy:
  * Precompute B[t] = Uo @ x[t-1] (a (2048, 28) matrix) and c on device.
    This phase is sharded over T across the 8 cores (each core handles 256
    timesteps of x, pre-transposed/interleaved on the host so the contraction
    dim D lands on SBUF partitions with one fully-contiguous DMA), then an
    AllGather shares the per-core (28 x 256) results + partial c sums.
  * Solve the recurrence by fixed-point (Jacobi) iteration:
        Y <- sigmoid(shift(Y) @ Wo.T + B + c)
    The map is a strong contraction (|sigmoid'| <= 1/4, ||Wo|| ~ 0.53;
    empirically the error floor is reached after 2-3 sweeps).
  * Iteration layout: t is split into 4 column groups of 512 stacked on
    partition blocks 28g..28g+27 (112 active partitions).  Each sweep is one
    three-matmul accumulation chain in fp32r (1 cycle/column on the PE):
        MM1: psum  = I112 @ bg                         (B term; bg pre-shifted)
        MM2: psum += blockdiag(Wo.T) @ YA[:, 0:512]    (shifted-y storage)
        MM3: psum += shiftblk(Wo.T) @ YA[:, 512:514]   (group boundary;
             col 513 is a permanent zero so the 2-col dst stays fp32r-legal)
    then one 112-lane sigmoid ACT with per-partition bias c writes
    YA[:, 1:513].  fp32r dst rules (start partition 0, even column count,
    8B alignment) hold by construction; masks are zero-padded host weights.

The kernel is self-contained: shapes/sharding are hardcoded.
"""

import numpy as np

import concourse.bass as bass
import concourse.mybir as mybir
import concourse.tile as tile
from concourse import bacc
from concourse import bass_utils

F32 = mybir.dt.float32
F32R = mybir.dt.float32r
BF16 = mybir.dt.bfloat16
AF = mybir.ActivationFunctionType

T, D, V = 2048, 1024, 28
N_CORES = 8
TC = T // N_CORES          # 256 timesteps per core in the B-precompute phase
G = 4                      # column groups in the iteration phase
S = T // G                 # 512 columns per group
P4 = G * V                 # 112 active partitions in the iteration phase
DCH = D // 128             # 8 contraction chunks
N_ITERS = 3                # fixed-point refinement sweeps (after the init sweep)
W2 = 64                    # padded [Uo;Co] output rows: Uo 0:28, Co 32:60
TH = TC + 2                # per-core timestep window incl. 2-col halo (even)

USE_F32R = True
USE_CC = True              # AllGather on; off = single-core-data debug mode


def build_body(nc, xt, w2t, wmm, eye, yg, n_iters=N_ITERS, tc=None,
               reps=1):
    """Emit the program. xt:(128, 8*256) x chunk, d-major interleaved;
    w2t:(1024,64) zero-padded [Uo;Co].T; wmm:(112, 3, 112) block weights
    ([.,0,.]=I112, [.,1,.]=blockdiag(Wo.T), [.,2,.]=boundary-shift(Wo.T));
    yg:(112,512) grouped output."""
    t = tc
    from contextlib import ExitStack
    ctx = ExitStack()
    sbp = ctx.enter_context(t.tile_pool(name="sb", bufs=1))
    pp = ctx.enter_context(t.tile_pool(name="pp", bufs=1, space="PSUM"))
    dp = ctx.enter_context(t.tile_pool(name="dp", bufs=2, space="DRAM"))

    MDT = F32R if USE_F32R else F32

    def st(shape, name, dt=F32):
        return sbp.tile(shape, dt, name=name, tag=name)

    xt_sb = st([128, 2, DCH, TH], "xt_sb", BF16)
    w2t_sb = st([128, 2, DCH, W2], "w2t_sb", BF16)
    wmm_sb = st([P4, 2, P4], "wmm_sb", MDT)
    eye_sb = st([P4, P4], "eye_sb", BF16)
    usb = st([W2, 2, TH], "usb", BF16)
    cpart = st([W2, 1], "cpart")
    cprt_bf = st([W2, 2], "cprt_bf", BF16)
    csb = st([P4, 2 * N_CORES], "csb", BF16)
    cbias = st([P4, 1], "cbias")
    bg = st([P4, 2, S], "bg", BF16)
    ya = st([P4, S + 2], "ya", MDT)
    yfin = st([P4, S], "yfin")
    dummy = st([1, 1], "dummy")

    upsum = pp.tile([W2, TH], F32, name="upsum", tag="upsum")
    psa = pp.tile([P4, S], F32, name="psa", tag="psa")
    psb = pp.tile([P4, S], F32, name="psb", tag="psb")

    # Early dummy sigmoid so the ACT table load happens off the critical path.
    nc.vector.memset(dummy[:, :], 0.0)
    nc.scalar.activation(out=dummy[:, :], in_=dummy[:, :], func=AF.Sigmoid)

    # one-time constants
    nc.sync.dma_start(wmm_sb[:, :, :], wmm)
    nc.sync.dma_start(eye_sb[:, :], eye)
    nc.sync.dma_start(w2t_sb[:, :, :, :],
                      w2t.rearrange("p (h c v) -> p h c v", h=2, c=DCH))
    nc.vector.memset(bg[:, :, :].bitcast(mybir.dt.uint16), 0)
    nc.vector.memset(ya[:, :].bitcast(F32), 0.0)

    prev_last = None
    for _rep in range(reps):
        prev_last = emit_rep(nc, t, dp, xt, yg, n_iters,
                             xt_sb, w2t_sb, wmm_sb, eye_sb, usb,
                             cpart, cprt_bf, csb, cbias, bg, ya, yfin,
                             upsum, psa, psb, prev_last)
    ctx.close()


def emit_rep(nc, t, dp, xt, yg, n_iters,
             xt_sb, w2t_sb, wmm_sb, eye_sb, usb, cpart, cprt_bf, csb,
             cbias, bg, ya, yfin, upsum, psa, psb, prev_last=None):
    from concourse.tile_rust import add_dep_helper
    MDT = F32R if USE_F32R else F32
    pay = dp.tile([V, 2 * TH + 2], BF16, name="pay", tag="pay")
    agout = dp.tile([V * N_CORES, 2 * TH + 2], BF16, name="agout",
                    tag="agout", addr_space="Shared")

    # ---------------- load x chunk (one fully-contiguous 1MB DMA) ----------
    xdma = nc.sync.dma_start(xt_sb[:, :, :, :],
                             xt.rearrange("p (h c t) -> p h c t", h=2, c=DCH))
    if prev_last is not None:
        add_dep_helper(xdma.ins, prev_last.ins,
                       reason="serialize reps for latency measurement")

    # -------- U = [Uo;Co] @ x_chunk.T  -> (64, 258), bf16 hi/lo split ------
    terms = [(0, 0), (0, 1), (1, 0)]   # (w half, x half); lo*lo dropped
    nmm = DCH * len(terms)
    i = 0
    for c in range(DCH):
        for hw, hx in terms:
            i += 1
            nc.tensor.matmul(
                upsum[:, :],
                lhsT=w2t_sb[:, hw, c, :],
                rhs=xt_sb[:, hx, c, :],
                start=(i == 1),
                stop=(i == nmm),
            )
    nc.vector.tensor_copy(usb[:, 0, :], upsum[:, :])
    nc.vector.tensor_tensor(usb[:, 1, :], upsum[:, :], usb[:, 0, :],
                            mybir.AluOpType.subtract)
    # partial c: row-sums of the Co part (own timesteps only, not the halo)
    nc.vector.tensor_reduce(
        out=cpart[32:32 + V, :], in_=upsum[32:32 + V, 2:TH],
        axis=mybir.AxisListType.X, op=mybir.AluOpType.add,
    )
    nc.vector.tensor_copy(cprt_bf[32:32 + V, 0:1], cpart[32:32 + V, :])
    nc.vector.tensor_tensor(cprt_bf[32:32 + V, 1:2], cpart[32:32 + V, :],
                            cprt_bf[32:32 + V, 0:1],
                            mybir.AluOpType.subtract)

    # ---------------- AllGather U chunks + partial c ----------------
    nc.sync.dma_start(pay[0:V, 0:2 * TH], usb[0:V, :, :])
    nc.sync.dma_start(pay[0:V, 2 * TH:2 * TH + 2], cprt_bf[32:32 + V, :])
    if USE_CC:
        nc.gpsimd.collective_compute(
            "AllGather",
            mybir.AluOpType.bypass,
            replica_groups=[list(range(N_CORES))],
            ins=[pay.opt()],
            outs=[agout.opt()],
        )
    else:
        nc.sync.dma_start(agout[0:V, :], pay[:, :])

    # ---------------- assemble grouped B and c ----------------
    # bg[28g+v, tau] = U[512g + tau - 1, v].  Core r's payload col j holds
    # U[256r - 2 + j] (2-col halo, core 0's halo is zero), so group g is
    # [core 2g cols 1:258 | core 2g+1 cols 2:257] with no boundary fixups.
    # Two full-112-partition DMAs: flat SBUF dst, (4,28,cols) DRAM src.
    # c = sum over cores of partial c; the (112 x 16) tile holds the hi/lo
    # partials replicated per partition group so one reduce yields the bias
    csrc = agout.opt().rearrange("(r p) f -> p r f", p=V)[0:V, :,
                                                          2 * TH:2 * TH + 2]
    for g in range(G):
        nc.sync.dma_start(csb[V * g:V * g + V, :], csrc)
    nc.vector.tensor_reduce(out=cbias[:, :], in_=csb[:, :],
                            axis=mybir.AxisListType.X, op=mybir.AluOpType.add)

    agv = agout.opt().rearrange("(r p) f -> r p f", p=V)
    for h in range(2):
        o = h * TH
        nc.sync.dma_start(bg[0:P4, h, 0:TC + 1],
                          agv[0:2 * G:2, :, o + 1:o + TH])
        nc.sync.dma_start(bg[0:P4, h, TC + 1:S],
                          agv[1:2 * G:2, :, o + 2:o + TC + 1])

    # ---------------- fixed-point iterations ----------------
    # YA[28g+v, j] stores y[512g + j - 1] for j in 1..512; col 0 and col 513
    # are permanent zeros (memset once).  psum col tau = z[512g + tau] before
    # the bias; ACT writes sigmoid(psum + c) into YA[:, 1:513].
    for k in range(n_iters + 1):
        ps = psa if k % 2 == 0 else psb
        for h in range(2):
            nc.tensor.matmul(
                ps[:, :],
                lhsT=eye_sb[:, :],
                rhs=bg[:, h, :],
                start=(h == 0), stop=(k == 0 and h == 1),
            )
        if k > 0:
            nc.tensor.matmul(
                ps[:, :],
                lhsT=wmm_sb[:, 0, :],
                rhs=ya[:, 0:S],
                start=False, stop=False,
            )
            nc.tensor.matmul(
                ps[:, 0:2],
                lhsT=wmm_sb[:, 1, :],
                rhs=ya[:, S:S + 2],
                start=False, stop=True,
            )
        if k < n_iters:
            nc.scalar.activation(out=ya[:, 1:S + 1], in_=ps[:, :],
                                 func=AF.Sigmoid, bias=cbias[:, 0:1],
                                 scale=1.0)
        else:
            nc.scalar.activation(out=yfin[:, :], in_=ps[:, :],
                                 func=AF.Sigmoid, bias=cbias[:, 0:1],
                                 scale=1.0)

    # ---------------- write grouped output ----------------
    return nc.sync.dma_start(yg, yfin[:, :])


_CACHED_NC = {}


def _get_nc(reps=1):
    if reps not in _CACHED_NC:
        nc = bacc.Bacc("TRN2", target_bir_lowering=False, debug=False,
                       num_devices=N_CORES)
        MDT = F32R if USE_F32R else F32
        xt = nc.dram_tensor("xt", [128, 2 * DCH * TH], BF16,
                            kind="ExternalInput")
        w2t = nc.dram_tensor("w2t", [128, 2 * DCH * W2], BF16,
                             kind="ExternalInput")
        wmm = nc.dram_tensor("wmm", [P4, 2, P4], MDT, kind="ExternalInput")
        eye = nc.dram_tensor("eye", [P4, P4], BF16, kind="ExternalInput")
        yg = nc.dram_tensor("yg", [P4, S], F32, kind="ExternalOutput")
        with tile.TileContext(nc) as t:
            build_body(nc, xt.ap(), w2t.ap(), wmm.ap(), eye.ap(), yg.ap(),
                       tc=t, reps=reps)
        nc.compile()
        _CACHED_NC[reps] = nc
    return _CACHED_NC[reps]


def _hilo(a):
    """Split fp32 array into (hi, lo) bf16 parts: a ~ hi + lo."""
    import ml_dtypes
    hi = a.astype(ml_dtypes.bfloat16)
    lo = (a - hi.astype(np.float32)).astype(ml_dtypes.bfloat16)
    return hi, lo


def make_in_maps(x, Uo, Co, Wo):
    import ml_dtypes
    xb = np.ascontiguousarray(np.asarray(x, np.float32)[0])        # (T, D)
    w2 = np.zeros((W2, D), np.float32)
    w2[0:V] = np.asarray(Uo, np.float32)
    w2[32:32 + V] = np.asarray(Co, np.float32)
    w2tf = np.ascontiguousarray(
        w2.T.reshape(DCH, 128, W2).transpose(1, 0, 2))             # (128,8,64)
    w2h, w2l = _hilo(w2tf)
    w2t = np.ascontiguousarray(
        np.stack([w2h, w2l], axis=1).reshape(128, 2 * DCH * W2))
    wot1 = np.ascontiguousarray(np.asarray(Wo, np.float32).T)      # (V, V)
    wmm = np.zeros((P4, 2, P4), np.float32)
    for g in range(G):
        wmm[V * g:V * g + V, 0, V * g:V * g + V] = wot1
        if g > 0:
            wmm[V * (g - 1):V * (g - 1) + V, 1, V * g:V * g + V] = wot1
    eye = np.eye(P4, dtype=ml_dtypes.bfloat16)
    in_maps = []
    for r in range(N_CORES):
        xh = np.zeros((TH, D), np.float32)                         # (258, D)
        lo = r * TC - 2
        xh[max(0, -lo):, :] = xb[max(0, lo):(r + 1) * TC, :]
        xc = np.ascontiguousarray(
            xh.T.reshape(DCH, 128, TH).transpose(1, 0, 2))         # (128,8,258)
        xhi, xlo = _hilo(xc)
        xi = np.ascontiguousarray(
            np.stack([xhi, xlo], axis=1).reshape(128, 2 * DCH * TH))
        in_maps.append({"xt": xi, "w2t": w2t, "wmm": wmm, "eye": eye})
    return in_maps


def unshard_output(yg):
    y = np.empty((T, V), np.float32)
    for g in range(G):
        y[g * S:(g + 1) * S, :] = yg[V * g:V * g + V, :].T
    return y[None]


def run(inputs, trace=False, reps=1, **kw):
    nc = _get_nc(reps)
    in_maps = make_in_maps(inputs["x"], inputs["Uo"], inputs["Co"],
                           inputs["Wo"])
    res = bass_utils.run_bass_kernel_spmd(
        nc, in_maps, core_ids=list(range(N_CORES)), trace=trace, **kw)
    return unshard_output(res.results[0]["yg"]), res


def kernel(**inputs):
    out, _ = run(inputs)
    return out



# revision 5
# speedup vs baseline: 1.9649x; 1.9649x over previous
"""Trainium2 Bass kernel for nn_CascadedAttention_76836964925817.

Math: the reference module's attention machinery is dead code — softmax over a
size-1 axis is identically 1, so `context = x[0].sum(axis=0)` is a constant
and the layer reduces to the 28-dim nonlinear recurrence

    y[t] = sigmoid(Wo @ y[t-1] + Uo @ x[t-1] + c),   c = Co @ sum_t x[t],
    y[-1] = 0, x[-1] := 0,

solved by Jacobi fixed-point sweeps (the map is a strong contraction:
|sigmoid'| <= 1/4, ||Wo|| ~ 0.53, plus heavy sigmoid saturation from the
large |c|; 3 sweeps reach the fp16 data floor of ~1e-3 rel).

Strategy (v2): **no collectives**.  Every core redundantly computes the
whole problem from an fp16 copy of x (4.2 MB, half the bytes/matmuls of a
bf16 hi/lo split; fp16's 11-bit mantissa keeps the c-path accurate); the
host reads core 0's output.  This removes the ~60us AllGather latency that
dominated the previous (T-sharded) version.

  * U-phase: x is d-major ((128, 8 chunks, 1+2048) fp16, one leading zero
    column so B's shift-by-one is just a column offset).  For each chunk,
    4 matmuls (lhsT = padded Uo chunk (128, 32)) write the four 512-col
    t-groups directly into ONE stacked psum bank at 32-partition strides
    (tile_position=(0, 32g)) — the bank then IS the shifted B matrix.
  * c-path (off critical path): per-chunk t-sums on Vector/GpSimd while
    the PE streams U, then 8 tiny accumulating matmuls against Co chunks,
    a copy + 4 small sbuf->sbuf DMAs to group-replicate the bias.
  * Sweeps: sweep 0 is a single ACT (sigmoid(B + c) straight from psum).
    Sweeps 1-2: blockdiag(Wo^T) matmul on the stacked YA (plus a 1-col
    group-boundary matmul), Vector add of B, ACT with bias=c.

The kernel is self-contained: shapes/sharding are hardcoded.
"""

import numpy as np

import concourse.bass as bass
import concourse.mybir as mybir
import concourse.tile as tile
from concourse import bacc
from concourse import bass_utils

F32 = mybir.dt.float32
F16 = mybir.dt.float16
U16 = mybir.dt.uint16
AF = mybir.ActivationFunctionType

T, D, V = 2048, 1024, 28
N_CORES = 8
G = 4                      # t-groups, stacked on partition blocks 32g..32g+27
S = T // G                 # 512 columns per group (= one psum bank)
DCH = D // 128             # 8 contraction chunks
CPC = T + 1                # x cols per chunk incl leading zero column
NSWEEP = 3                 # Jacobi sweeps (sweep 0 is ACT-only)


def build_body(nc, xt, w2t, cot, wmm, yg, tc):
    from contextlib import ExitStack
    ctx = ExitStack()
    sbp = ctx.enter_context(tc.tile_pool(name="sb", bufs=1))
    pp = ctx.enter_context(tc.tile_pool(name="pp", bufs=1, space="PSUM"))

    def st(shape, name, dt=F32):
        return sbp.tile(shape, dt, name=name, tag=name)

    xt_sb = st([128, DCH, CPC], "xt_sb", F16)
    w2t_sb = st([128, DCH, 32], "w2t_sb", F16)
    cot_sb = st([128, DCH, V], "cot_sb", F16)
    wmm_sb = st([128, 2, 128], "wmm_sb", F16)
    ya = st([128, S + 1], "ya", F16)
    bsb = st([128, S], "bsb", F16)
    zsb = st([128, S], "zsb", F16)
    scol = st([128, DCH], "scol")
    scol16 = st([128, DCH], "scol16", F16)
    csb = st([V, 1], "csb")
    cbias = st([128, 1], "cbias")
    yfin = st([128, S], "yfin")
    dummy = st([1, 1], "dummy")

    psB = pp.tile([128, S], F32, name="psB", tag="psB")
    z1 = pp.tile([128, S], F32, name="z1", tag="z1")
    z2 = pp.tile([128, S], F32, name="z2", tag="z2")
    cps = pp.tile([V, 1], F32, name="cps", tag="cps")

    # Early dummy sigmoid so the ACT table load happens off the critical path.
    nc.vector.memset(dummy[:, :], 0.0)
    nc.scalar.activation(out=dummy[:, :], in_=dummy[:, :], func=AF.Sigmoid)
    nc.vector.memset(ya[:, :].bitcast(U16), 0)
    nc.vector.memset(cbias[:, :], 0.0)

    # one-time constants
    nc.sync.dma_start(w2t_sb[:, :, :], w2t.rearrange("p (c w) -> p c w", c=DCH))
    nc.sync.dma_start(cot_sb[:, :, :], cot.rearrange("p (c v) -> p c v", c=DCH))
    nc.sync.dma_start(wmm_sb[:, :, :], wmm.rearrange("p (h q) -> p h q", h=2))

    # x chunks (8 x 513KB)
    xv = xt.rearrange("p (c t) -> p c t", c=DCH)
    for c in range(DCH):
        nc.sync.dma_start(xt_sb[:, c, :], xv[:, c, :])

    # ---- U-phase: psB[32g+v, tau] = sum_d Uo[v,d] x[512g+tau-1, d] ----
    # c-path t-sums ride on Vector/GpSimd while the PE streams.
    for c in range(DCH):
        nc.vector.tensor_reduce(out=scol[:, c:c + 1], in_=xt_sb[:, c, :],
                                axis=mybir.AxisListType.X,
                                op=mybir.AluOpType.add)
    for c in range(DCH):
        if c == DCH - 1:
            # c = Co @ s: 8 tiny accumulating matmuls, scheduled before the
            # last chunk's group matmuls so the bias is ready off-path.
            nc.vector.tensor_copy(scol16[:, :], scol[:, :])
            for cc in range(DCH):
                nc.tensor.matmul(
                    cps[:, :], lhsT=cot_sb[:, cc, :],
                    rhs=scol16[:, cc:cc + 1],
                    start=(cc == 0), stop=(cc == DCH - 1),
                    skip_group_check=True,
                )
            nc.vector.tensor_copy(csb[:, :], cps[:, :])
            for g in range(G):
                nc.sync.dma_start(cbias[32 * g:32 * g + V, :], csb[:, :])
        for g in range(G):
            nc.tensor.matmul(
                psB[32 * g:32 * g + 32, :],
                lhsT=w2t_sb[:, c, :],
                rhs=xt_sb[:, c, S * g:S * g + S],
                start=(c == 0), stop=(c == DCH - 1),
                tile_position=(0, 32 * g),
                skip_group_check=True,
            )

    # fp16 copy of B for sweeps 1+ (sweep 0 reads the psum directly)
    nc.vector.tensor_copy(bsb[:, :], psB[:, :])

    # ---- Jacobi sweeps ----
    # YA[32g+v, j]: j=0 boundary col (zero; block boundaries flow through the
    # wmm[:,1,:] shift matmul), j>=1 holds y[512g+j-1].
    for k in range(NSWEEP):
        if k == 0:
            nc.scalar.activation(out=ya[:, 1:S + 1], in_=psB[:, :],
                                 func=AF.Sigmoid, bias=cbias[:, 0:1],
                                 scale=1.0)
            continue
        z = z1 if k % 2 == 1 else z2
        nc.tensor.matmul(z[:, :], lhsT=wmm_sb[:, 0, :], rhs=ya[:, 0:S],
                         start=True, stop=False, skip_group_check=True)
        nc.tensor.matmul(z[:, 0:1], lhsT=wmm_sb[:, 1, :], rhs=ya[:, S:S + 1],
                         start=False, stop=True, skip_group_check=True)
        nc.vector.tensor_tensor(zsb[:, :], z[:, :], bsb[:, :],
                                mybir.AluOpType.add)
        if k < NSWEEP - 1:
            nc.scalar.activation(out=ya[:, 1:S + 1], in_=zsb[:, :],
                                 func=AF.Sigmoid, bias=cbias[:, 0:1],
                                 scale=1.0)
        else:
            nc.scalar.activation(out=yfin[:, :], in_=zsb[:, :],
                                 func=AF.Sigmoid, bias=cbias[:, 0:1],
                                 scale=1.0)

    nc.sync.dma_start(yg, yfin[:, :])
    ctx.close()


_CACHED_NC = {}


def _get_nc():
    if "nc" not in _CACHED_NC:
        nc = bacc.Bacc("TRN2", target_bir_lowering=False, debug=False,
                       num_devices=N_CORES)
        xt = nc.dram_tensor("xt", [128, DCH * CPC], F16, kind="ExternalInput")
        w2t = nc.dram_tensor("w2t", [128, DCH * 32], F16, kind="ExternalInput")
        cot = nc.dram_tensor("cot", [128, DCH * V], F16, kind="ExternalInput")
        wmm = nc.dram_tensor("wmm", [128, 2 * 128], F16, kind="ExternalInput")
        yg = nc.dram_tensor("yg", [128, S], F32, kind="ExternalOutput")
        with tile.TileContext(nc) as tc:
            build_body(nc, xt.ap(), w2t.ap(), cot.ap(), wmm.ap(), yg.ap(),
                       tc)
        nc.compile()
        _CACHED_NC["nc"] = nc
    return _CACHED_NC["nc"]


def make_in_maps(x, Uo, Co, Wo):
    xb = np.asarray(x, np.float32)[0]                              # (T, D)
    xc = np.zeros((128, DCH, CPC), np.float16)
    xc[:, :, 1:] = xb.T.reshape(DCH, 128, T).transpose(1, 0, 2)
    xc = np.ascontiguousarray(xc.reshape(128, DCH * CPC))

    w2 = np.zeros((32, D), np.float32)
    w2[0:V] = np.asarray(Uo, np.float32)
    w2t = np.ascontiguousarray(
        w2.T.reshape(DCH, 128, 32).transpose(1, 0, 2)
        .reshape(128, DCH * 32)).astype(np.float16)

    cot = np.ascontiguousarray(
        np.asarray(Co, np.float32).T.reshape(DCH, 128, V).transpose(1, 0, 2)
        .reshape(128, DCH * V)).astype(np.float16)

    wot = np.asarray(Wo, np.float32).T                             # (v, w)
    wmm = np.zeros((128, 2, 128), np.float16)
    for g in range(G):
        wmm[32 * g:32 * g + V, 0, 32 * g:32 * g + V] = wot
        if g > 0:
            wmm[32 * (g - 1):32 * (g - 1) + V, 1, 32 * g:32 * g + V] = wot
    wmm = np.ascontiguousarray(wmm.reshape(128, 2 * 128))

    m = {"xt": xc, "w2t": w2t, "cot": cot, "wmm": wmm}
    return [m] * N_CORES


def unshard_output(yg):
    y = np.empty((T, V), np.float32)
    for g in range(G):
        y[g * S:(g + 1) * S, :] = yg[32 * g:32 * g + V, :].T
    return y[None]


def run(inputs, trace=False, **kw):
    nc = _get_nc()
    in_maps = make_in_maps(inputs["x"], inputs["Uo"], inputs["Co"],
                           inputs["Wo"])
    res = bass_utils.run_bass_kernel_spmd(
        nc, in_maps, core_ids=list(range(N_CORES)), trace=trace, **kw)
    return unshard_output(res.results[0]["yg"]), res


def kernel(**inputs):
    out, _ = run(inputs)
    return out


# revision 7
# speedup vs baseline: 1.9787x; 1.0070x over previous
"""Trainium2 Bass kernel for nn_CascadedAttention_76836964925817.

Math: the reference module's attention machinery is dead code — softmax over a
size-1 axis is identically 1, so `context = x[0].sum(axis=0)` is a constant
and the layer reduces to the 28-dim nonlinear recurrence

    y[t] = sigmoid(Wo @ y[t-1] + Uo @ x[t-1] + c),   c = Co @ sum_t x[t],
    y[-1] = 0, x[-1] := 0,

solved by Jacobi fixed-point sweeps (the map is a strong contraction:
|sigmoid'| <= 1/4, ||Wo|| ~ 0.53, plus heavy sigmoid saturation from the
large |c|; 3 sweeps reach the fp16 data floor of ~1e-3 rel).

Strategy (v2): **no collectives**.  Every core redundantly computes the
whole problem from an fp16 copy of x (4.2 MB, half the bytes/matmuls of a
bf16 hi/lo split; fp16's 11-bit mantissa keeps the c-path accurate); the
host reads core 0's output.  This removes the ~60us AllGather latency that
dominated the previous (T-sharded) version.

  * U-phase: x is d-major ((128, 8 chunks, 1+2048) fp16, one leading zero
    column so B's shift-by-one is just a column offset).  For each chunk,
    4 matmuls (lhsT = padded Uo chunk (128, 32)) write the four 512-col
    t-groups directly into ONE stacked psum bank at 32-partition strides
    (tile_position=(0, 32g)) — the bank then IS the shifted B matrix.
  * c-path (off critical path): per-chunk t-sums on Vector/GpSimd while
    the PE streams U, then 8 tiny accumulating matmuls against Co chunks,
    a copy + 4 small sbuf->sbuf DMAs to group-replicate the bias.
  * Sweeps: sweep 0 is a single ACT (sigmoid(B + c) straight from psum).
    Sweeps 1-2: blockdiag(Wo^T) matmul on the stacked YA (plus a 1-col
    group-boundary matmul), Vector add of B, ACT with bias=c.

The kernel is self-contained: shapes/sharding are hardcoded.
"""

import numpy as np

import concourse.bass as bass
import concourse.mybir as mybir
import concourse.tile as tile
from concourse import bacc
from concourse import bass_utils

F32 = mybir.dt.float32
F16 = mybir.dt.float16
U16 = mybir.dt.uint16
AF = mybir.ActivationFunctionType

T, D, V = 2048, 1024, 28
N_CORES = 8
G = 4                      # t-groups, stacked on partition blocks 32g..32g+27
S = T // G                 # 512 columns per group (= one psum bank)
DCH = D // 128             # 8 contraction chunks
CPC = T + 1                # x cols per chunk incl leading zero column
NSWEEP = 3                 # Jacobi sweeps (sweep 0 is ACT-only)


def build_body(nc, xt, w2t, cot, wmm, yg, tc):
    from contextlib import ExitStack
    ctx = ExitStack()
    sbp = ctx.enter_context(tc.tile_pool(name="sb", bufs=1))
    pp = ctx.enter_context(tc.tile_pool(name="pp", bufs=1, space="PSUM"))

    def st(shape, name, dt=F32):
        return sbp.tile(shape, dt, name=name, tag=name)

    xt_sb = st([128, DCH, CPC], "xt_sb", F16)
    w2t_sb = st([128, DCH, 32], "w2t_sb", F16)
    cot_sb = st([128, DCH, V], "cot_sb", F16)
    wmm_sb = st([128, 2, 128], "wmm_sb", F16)
    ya = st([128, S + 1], "ya", F16)
    bsb = st([128, S], "bsb", F16)
    zsb = st([128, S], "zsb", F16)
    scol = st([128, DCH], "scol")
    scol16 = st([128, DCH], "scol16", F16)
    spart = st([128, 64], "spart")
    csb = st([V, 1], "csb")
    cbias = st([128, 1], "cbias")
    yfin = st([128, S], "yfin")
    dummy = st([1, 1], "dummy")

    psB = pp.tile([128, S], F32, name="psB", tag="psB")
    z1 = pp.tile([128, S], F32, name="z1", tag="z1")
    z2 = pp.tile([128, S], F32, name="z2", tag="z2")
    cps = pp.tile([V, 1], F32, name="cps", tag="cps")

    # Early dummy sigmoid so the ACT table load happens off the critical path.
    nc.vector.memset(dummy[:, :], 0.0)
    nc.scalar.activation(out=dummy[:, :], in_=dummy[:, :], func=AF.Sigmoid)
    nc.vector.memset(ya[:, :].bitcast(U16), 0)
    nc.vector.memset(cbias[:, :], 0.0)

    # one-time constants
    nc.sync.dma_start(w2t_sb[:, :, :], w2t.rearrange("p (c w) -> p c w", c=DCH))
    nc.sync.dma_start(cot_sb[:, :, :], cot.rearrange("p (c v) -> p c v", c=DCH))
    nc.sync.dma_start(wmm_sb[:, :, :], wmm.rearrange("p (h q) -> p h q", h=2))

    # x chunks (8 x 513KB)
    xv = xt.rearrange("p (c t) -> p c t", c=DCH)
    for c in range(DCH):
        nc.sync.dma_start(xt_sb[:, c, :], xv[:, c, :])

    # ---- U-phase: psB[32g+v, tau] = sum_d Uo[v,d] x[512g+tau-1, d] ----
    # c-path t-sums ride on Vector/GpSimd while the PE streams.
    # Two-stage t-sums: fp16 chains of 32, then a pure-fp32 reduction, so the
    # result does not depend on the DVE's internal fp16 accumulator width.
    for c in range(DCH):
        nc.vector.tensor_reduce(
            out=spart[:, :],
            in_=xt_sb[:, c, 1:].rearrange("p (a b) -> p a b", a=64),
            axis=mybir.AxisListType.X, op=mybir.AluOpType.add)
        nc.vector.tensor_reduce(out=scol[:, c:c + 1], in_=spart[:, :],
                                axis=mybir.AxisListType.X,
                                op=mybir.AluOpType.add)
    for c in range(DCH):
        if c == DCH - 1:
            # c = Co @ s: 8 tiny accumulating matmuls, scheduled before the
            # last chunk's group matmuls so the bias is ready off-path.
            nc.vector.tensor_copy(scol16[:, :], scol[:, :])
            for cc in range(DCH):
                nc.tensor.matmul(
                    cps[:, :], lhsT=cot_sb[:, cc, :],
                    rhs=scol16[:, cc:cc + 1],
                    start=(cc == 0), stop=(cc == DCH - 1),
                    skip_group_check=True,
                )
            nc.vector.tensor_copy(csb[:, :], cps[:, :])
            for g in range(G):
                nc.sync.dma_start(cbias[32 * g:32 * g + V, :], csb[:, :])
        for g in range(G):
            nc.tensor.matmul(
                psB[32 * g:32 * g + 32, :],
                lhsT=w2t_sb[:, c, :],
                rhs=xt_sb[:, c, S * g:S * g + S],
                start=(c == 0), stop=(c == DCH - 1),
                tile_position=(0, 32 * g),
                skip_group_check=True,
            )

    # fp16 copy of B for sweeps 1+ (sweep 0 reads the psum directly)
    nc.vector.tensor_copy(bsb[:, :], psB[:, :])

    # ---- Jacobi sweeps ----
    # YA[32g+v, j]: j=0 boundary col (zero; block boundaries flow through the
    # wmm[:,1,:] shift matmul), j>=1 holds y[512g+j-1].
    for k in range(NSWEEP):
        if k == 0:
            nc.scalar.activation(out=ya[:, 1:S + 1], in_=psB[:, :],
                                 func=AF.Sigmoid, bias=cbias[:, 0:1],
                                 scale=1.0)
            continue
        z = z1 if k % 2 == 1 else z2
        nc.tensor.matmul(z[:, :], lhsT=wmm_sb[:, 0, :], rhs=ya[:, 0:S],
                         start=True, stop=False, skip_group_check=True)
        nc.tensor.matmul(z[:, 0:1], lhsT=wmm_sb[:, 1, :], rhs=ya[:, S:S + 1],
                         start=False, stop=True, skip_group_check=True)
        nc.vector.tensor_tensor(zsb[:, :], z[:, :], bsb[:, :],
                                mybir.AluOpType.add)
        if k < NSWEEP - 1:
            nc.scalar.activation(out=ya[:, 1:S + 1], in_=zsb[:, :],
                                 func=AF.Sigmoid, bias=cbias[:, 0:1],
                                 scale=1.0)
        else:
            nc.scalar.activation(out=yfin[:, :], in_=zsb[:, :],
                                 func=AF.Sigmoid, bias=cbias[:, 0:1],
                                 scale=1.0)

    nc.sync.dma_start(yg, yfin[:, :])
    ctx.close()


_CACHED_NC = {}


def _get_nc():
    if "nc" not in _CACHED_NC:
        nc = bacc.Bacc("TRN2", target_bir_lowering=False, debug=False,
                       num_devices=N_CORES)
        xt = nc.dram_tensor("xt", [128, DCH * CPC], F16, kind="ExternalInput")
        w2t = nc.dram_tensor("w2t", [128, DCH * 32], F16, kind="ExternalInput")
        cot = nc.dram_tensor("cot", [128, DCH * V], F16, kind="ExternalInput")
        wmm = nc.dram_tensor("wmm", [128, 2 * 128], F16, kind="ExternalInput")
        yg = nc.dram_tensor("yg", [128, S], F32, kind="ExternalOutput")
        with tile.TileContext(nc) as tc:
            build_body(nc, xt.ap(), w2t.ap(), cot.ap(), wmm.ap(), yg.ap(),
                       tc)
        nc.compile()
        _CACHED_NC["nc"] = nc
    return _CACHED_NC["nc"]


def make_in_maps(x, Uo, Co, Wo):
    xb = np.asarray(x, np.float32)[0]                              # (T, D)
    xc = np.zeros((128, DCH, CPC), np.float16)
    xc[:, :, 1:] = xb.T.reshape(DCH, 128, T).transpose(1, 0, 2)
    xc = np.ascontiguousarray(xc.reshape(128, DCH * CPC))

    w2 = np.zeros((32, D), np.float32)
    w2[0:V] = np.asarray(Uo, np.float32)
    w2t = np.ascontiguousarray(
        w2.T.reshape(DCH, 128, 32).transpose(1, 0, 2)
        .reshape(128, DCH * 32)).astype(np.float16)

    cot = np.ascontiguousarray(
        np.asarray(Co, np.float32).T.reshape(DCH, 128, V).transpose(1, 0, 2)
        .reshape(128, DCH * V)).astype(np.float16)

    wot = np.asarray(Wo, np.float32).T                             # (v, w)
    wmm = np.zeros((128, 2, 128), np.float16)
    for g in range(G):
        wmm[32 * g:32 * g + V, 0, 32 * g:32 * g + V] = wot
        if g > 0:
            wmm[32 * (g - 1):32 * (g - 1) + V, 1, 32 * g:32 * g + V] = wot
    wmm = np.ascontiguousarray(wmm.reshape(128, 2 * 128))

    m = {"xt": xc, "w2t": w2t, "cot": cot, "wmm": wmm}
    return [m] * N_CORES


def unshard_output(yg):
    y = np.empty((T, V), np.float32)
    for g in range(G):
        y[g * S:(g + 1) * S, :] = yg[32 * g:32 * g + V, :].T
    return y[None]


def run(inputs, trace=False, **kw):
    nc = _get_nc()
    in_maps = make_in_maps(inputs["x"], inputs["Uo"], inputs["Co"],
                           inputs["Wo"])
    res = bass_utils.run_bass_kernel_spmd(
        nc, in_maps, core_ids=list(range(N_CORES)), trace=trace, **kw)
    return unshard_output(res.results[0]["yg"]), res


def kernel(**inputs):
    out, _ = run(inputs)
    return out


# revision 8
# speedup vs baseline: 2.7180x; 1.3736x over previous
"""Trainium2 Bass kernel for nn_CascadedAttention_76836964925817.

Math: the reference module's attention machinery is dead code — softmax over a
size-1 axis is identically 1, so `context = x[0].sum(axis=0)` is a constant
and the layer reduces to the 28-dim nonlinear recurrence

    y[t] = sigmoid(Wo @ y[t-1] + Uo @ x[t-1] + c),   c = Co @ sum_t x[t],
    y[-1] = 0, x[-1] := 0,

solved by Jacobi fixed-point sweeps (the map is a strong contraction:
|sigmoid'| <= 1/4, ||Wo|| ~ 0.53, plus heavy sigmoid saturation from the
large |c|; 3 sweeps reach the fp16 data floor of ~1e-3 rel).

Strategy: **no collectives**.  Every core redundantly computes the whole
problem from an fp16 copy of x (4.2 MB); the host reads core 0's output.
This removes the ~60us AllGather latency that dominated the T-sharded
variant.

  * x is d-major ((128, 8 chunks, 1+2048) fp16, one leading zero column so
    B's shift-by-one is a column offset), streamed in 8 chunk DMAs split
    across the two HWDGE queues (SP + Activation) so consumers start early.
  * U-phase: per chunk, 4 matmuls (lhsT = padded Uo chunk (128, 32)) write
    the four 512-col t-groups into ONE stacked psum bank at 32-partition
    strides (tile_position=(0, 32g)) — the bank then IS the shifted B.
  * c-path (off critical path): per-chunk t-sums, even chunks on Vector
    (two-stage reduce, fp16 chains of 32 then fp32), odd chunks on Scalar
    (Identity activation with fp32 accum_out); then 8 tiny accumulating
    matmuls against group-replicated Co weights (128-wide lhsT) put the
    full 128-partition bias in psum; one Vector copy lands it in SBUF.
  * Sweeps: sweep 0 is one ACT (sigmoid(B + c) straight from the U psum).
    The two z psum banks are pre-loaded with B (eye matmuls, off-path);
    sweeps 1-2 accumulate blockdiag(Wo^T) @ YA (plus a 1-col group-boundary
    matmul) on top and ACT with bias=c.

The kernel is self-contained: shapes/sharding are hardcoded.
"""

import numpy as np

import concourse.bass as bass
import concourse.mybir as mybir
import concourse.tile as tile
from concourse import bacc
from concourse import bass_utils

F32 = mybir.dt.float32
F16 = mybir.dt.float16
U16 = mybir.dt.uint16
AF = mybir.ActivationFunctionType

T, D, V = 2048, 1024, 28
N_CORES = 8
G = 4                      # t-groups, stacked on partition blocks 32g..32g+27
S = T // G                 # 512 columns per group (= one psum bank)
DCH = D // 128             # 8 contraction chunks
CPC = T + 1                # x cols per chunk incl leading zero column
NSWEEP = 3                 # Jacobi sweeps (sweep 0 is ACT-only)


def build_body(nc, xt, w2t, cot4, wmm, eye, yg, tc):
    from contextlib import ExitStack
    ctx = ExitStack()
    sbp = ctx.enter_context(tc.tile_pool(name="sb", bufs=1))
    pp = ctx.enter_context(tc.tile_pool(name="pp", bufs=1, space="PSUM"))

    def st(shape, name, dt=F32):
        return sbp.tile(shape, dt, name=name, tag=name)

    xt_sb = st([128, DCH, CPC], "xt_sb", F16)
    w2t_sb = st([128, DCH, 32], "w2t_sb", F16)
    cot_sb = st([128, DCH, 128], "cot_sb", F16)
    wmm_sb = st([128, 2, 128], "wmm_sb", F16)
    eye_sb = st([128, 128], "eye_sb", F16)
    ya = st([128, S + 1], "ya", F16)
    bsb = st([128, S], "bsb", F16)
    scr = st([128, T], "scr", F16)
    scol = st([128, DCH], "scol")
    scol16 = st([128, DCH], "scol16", F16)
    spart = st([128, 64], "spart")
    cbias = st([128, 1], "cbias")
    yfin = st([128, S], "yfin", F16)
    dummy = st([1, 1], "dummy")

    psB = pp.tile([128, S], F32, name="psB", tag="psB")
    z1 = pp.tile([128, S], F32, name="z1", tag="z1")
    z2 = pp.tile([128, S], F32, name="z2", tag="z2")
    cps = pp.tile([128, 1], F32, name="cps", tag="cps")

    # Early dummy sigmoid so the ACT table load happens off the critical path.
    nc.vector.memset(dummy[:, :], 0.0)
    nc.scalar.activation(out=dummy[:, :], in_=dummy[:, :], func=AF.Sigmoid)
    nc.vector.memset(ya[:, :].bitcast(U16), 0)

    # constants: w2t first on the SP queue (gates the PE), the rest on the
    # GpSimd software queue (needed later)
    nc.sync.dma_start(w2t_sb[:, :, :], w2t.rearrange("p (c w) -> p c w", c=DCH))
    nc.gpsimd.dma_start(cot_sb[:, :, :],
                        cot4.rearrange("p (c w) -> p c w", c=DCH))
    nc.gpsimd.dma_start(wmm_sb[:, :, :], wmm.rearrange("p (h q) -> p h q", h=2))
    nc.gpsimd.dma_start(eye_sb[:, :], eye)

    # x chunks alternate between the two HWDGE queues
    xv = xt.rearrange("p (c t) -> p c t", c=DCH)
    for c in range(DCH):
        eng = nc.sync if c % 2 == 0 else nc.scalar
        eng.dma_start(xt_sb[:, c, :], xv[:, c, :])

    # ---- c-path t-sums (off the PE): even chunks Vector, odd Scalar ----
    for c in range(DCH):
        if c % 2 == 0:
            nc.vector.tensor_reduce(
                out=spart[:, :],
                in_=xt_sb[:, c, 1:].rearrange("p (a b) -> p a b", a=64),
                axis=mybir.AxisListType.X, op=mybir.AluOpType.add)
            nc.vector.tensor_reduce(out=scol[:, c:c + 1], in_=spart[:, :],
                                    axis=mybir.AxisListType.X,
                                    op=mybir.AluOpType.add)
        else:
            nc.scalar.activation(out=scr[:, :], in_=xt_sb[:, c, 1:],
                                 func=AF.Identity,
                                 accum_out=scol[:, c:c + 1])

    # ---- U-phase: psB[32g+v, tau] = sum_d Uo[v,d] x[512g+tau-1, d] ----
    for c in range(DCH):
        if c == DCH - 1:
            # c = Co @ s via group-replicated Co weights -> full-partition
            # bias in one psum column; scheduled before the last chunk's
            # group matmuls so the bias is ready off-path.
            nc.vector.tensor_copy(scol16[:, :], scol[:, :])
            for cc in range(DCH):
                nc.tensor.matmul(
                    cps[:, :], lhsT=cot_sb[:, cc, :],
                    rhs=scol16[:, cc:cc + 1],
                    start=(cc == 0), stop=(cc == DCH - 1),
                    skip_group_check=True,
                )
            nc.vector.tensor_copy(cbias[:, :], cps[:, :])
        for g in range(G):
            nc.tensor.matmul(
                psB[32 * g:32 * g + 32, :],
                lhsT=w2t_sb[:, c, :],
                rhs=xt_sb[:, c, S * g:S * g + S],
                start=(c == 0), stop=(c == DCH - 1),
                tile_position=(0, 32 * g),
                skip_group_check=True,
            )

    # fp16 copy of B; pre-load the sweep psum banks with it (PE idle then)
    nc.vector.tensor_copy(bsb[:, :], psB[:, :])
    for z in (z1, z2):
        nc.tensor.matmul(z[:, :], lhsT=eye_sb[:, :], rhs=bsb[:, :],
                         start=True, stop=False, skip_group_check=True)

    # ---- Jacobi sweeps ----
    # YA[32g+v, j]: j=0 boundary col (zero; block boundaries flow through the
    # wmm[:,1,:] shift matmul), j>=1 holds y[512g+j-1].
    for k in range(NSWEEP):
        if k == 0:
            nc.scalar.activation(out=ya[:, 1:S + 1], in_=psB[:, :],
                                 func=AF.Sigmoid, bias=cbias[:, 0:1],
                                 scale=1.0)
            continue
        z = z1 if k % 2 == 1 else z2
        nc.tensor.matmul(z[:, :], lhsT=wmm_sb[:, 0, :], rhs=ya[:, 0:S],
                         start=False, stop=False, skip_group_check=True)
        nc.tensor.matmul(z[:, 0:1], lhsT=wmm_sb[:, 1, :], rhs=ya[:, S:S + 1],
                         start=False, stop=True, skip_group_check=True)
        out = ya[:, 1:S + 1] if k < NSWEEP - 1 else yfin[:, :]
        nc.scalar.activation(out=out, in_=z[:, :], func=AF.Sigmoid,
                             bias=cbias[:, 0:1], scale=1.0)

    nc.sync.dma_start(yg, yfin[:, :])
    ctx.close()


_CACHED_NC = {}


def _get_nc():
    if "nc" not in _CACHED_NC:
        nc = bacc.Bacc("TRN2", target_bir_lowering=False, debug=False,
                       num_devices=N_CORES)
        xt = nc.dram_tensor("xt", [128, DCH * CPC], F16, kind="ExternalInput")
        w2t = nc.dram_tensor("w2t", [128, DCH * 32], F16, kind="ExternalInput")
        cot4 = nc.dram_tensor("cot4", [128, DCH * 128], F16,
                              kind="ExternalInput")
        wmm = nc.dram_tensor("wmm", [128, 2 * 128], F16, kind="ExternalInput")
        eye = nc.dram_tensor("eye", [128, 128], F16, kind="ExternalInput")
        yg = nc.dram_tensor("yg", [128, S], F16, kind="ExternalOutput")
        with tile.TileContext(nc) as tc:
            build_body(nc, xt.ap(), w2t.ap(), cot4.ap(), wmm.ap(), eye.ap(),
                       yg.ap(), tc)
        nc.compile()
        _CACHED_NC["nc"] = nc
    return _CACHED_NC["nc"]


def make_in_maps(x, Uo, Co, Wo):
    xb = np.asarray(x, np.float32)[0]                              # (T, D)
    xc = np.zeros((128, DCH, CPC), np.float16)
    xc[:, :, 1:] = xb.T.reshape(DCH, 128, T).transpose(1, 0, 2)
    xc = np.ascontiguousarray(xc.reshape(128, DCH * CPC))

    w2 = np.zeros((32, D), np.float32)
    w2[0:V] = np.asarray(Uo, np.float32)
    w2t = np.ascontiguousarray(
        w2.T.reshape(DCH, 128, 32).transpose(1, 0, 2)
        .reshape(128, DCH * 32)).astype(np.float16)

    c4 = np.zeros((128, D), np.float32)                            # (4*32, D)
    for g in range(G):
        c4[32 * g:32 * g + V] = np.asarray(Co, np.float32)
    cot4 = np.ascontiguousarray(
        c4.T.reshape(DCH, 128, 128).transpose(1, 0, 2)
        .reshape(128, DCH * 128)).astype(np.float16)

    wot = np.asarray(Wo, np.float32).T                             # (v, w)
    wmm = np.zeros((128, 2, 128), np.float16)
    for g in range(G):
        wmm[32 * g:32 * g + V, 0, 32 * g:32 * g + V] = wot
        if g > 0:
            wmm[32 * (g - 1):32 * (g - 1) + V, 1, 32 * g:32 * g + V] = wot
    wmm = np.ascontiguousarray(wmm.reshape(128, 2 * 128))

    eye = np.eye(128, dtype=np.float16)

    m = {"xt": xc, "w2t": w2t, "cot4": cot4, "wmm": wmm, "eye": eye}
    return [m] * N_CORES


def unshard_output(yg):
    y = np.empty((T, V), np.float32)
    for g in range(G):
        y[g * S:(g + 1) * S, :] = yg[32 * g:32 * g + V, :].astype(np.float32).T
    return y[None]


def run(inputs, trace=False, **kw):
    nc = _get_nc()
    in_maps = make_in_maps(inputs["x"], inputs["Uo"], inputs["Co"],
                           inputs["Wo"])
    res = bass_utils.run_bass_kernel_spmd(
        nc, in_maps, core_ids=list(range(N_CORES)), trace=trace, **kw)
    return unshard_output(res.results[0]["yg"]), res


def kernel(**inputs):
    out, _ = run(inputs)
    return out
